# revision 1
# baseline (speedup 1.0000x reference)
"""Local (windowed) attention kernel for Trainium2, sequence-parallel over 8 NeuronCores.

Reference computation (fp32):
    qkv = x @ w_qkv ; q,k,v split, reshaped to (head, window, 128, 64)
    k,v get a 1-window zero-padded lookback -> (head, window, 256, 64)
    sim = q @ k.T * d^-0.5, causal-banded mask, softmax, out = attn @ v
    y = out @ w_out + b_out

Sharding: 128 windows of 128 tokens -> 16 windows per core, plus a 128-row
halo of x from the previous core (zeros for core 0, which exactly reproduces
the reference's zero-pad lookback including its effect on the softmax
denominator). No inter-core communication.

Device dataflow (per core, all bf16 matmuls accumulating in fp32):
  xT (host-pretransposed, [1024, 2176]) and w_qkv stream in; qkT = w_q/k.T @ xT
  keeps features on partitions, v = xT.T @ w_v keeps tokens on partitions with
  a ones-column appended per head (so attn@v also yields the softmax
  denominator for free). Scores are computed transposed (pT[j, i]) so that
  attn@v needs no on-device transposes and its output lands directly as the
  stationary operand of the output projection. Softmax skips max-subtraction
  (logits are ~N(0, 0.4); exp is safe in fp32).
"""

import sys

sys.path.insert(0, "/opt/trn_rl_repo")

import numpy as np
import ml_dtypes

import concourse.bass as bass
import concourse.mybir as mybir
import concourse.tile as tile
from concourse import bacc
from concourse.bass_utils import run_bass_kernel_spmd

BF16 = mybir.dt.bfloat16
F32 = mybir.dt.float32

N = 16384
DIM = 1024
HEADS = 8
DHEAD = 64
WSZ = 128
NCORES = 8
R = N // NCORES            # 2048 own rows per core
T = R + WSZ                # 2176 rows incl. halo
NW = R // WSZ              # 16 own windows
DK = DIM // 128            # 8 contraction chunks
P = 128
SCALE = DHEAD ** -0.5

_CACHE = {}


def _build():
    nc = bacc.Bacc()
    xT_d = nc.declare_dram_parameter("xT", [DIM, T], BF16, isOutput=False)
    wqkv_d = nc.declare_dram_parameter("wqkv", [DIM, 3 * HEADS * DHEAD], BF16, isOutput=False)
    wout_d = nc.declare_dram_parameter("wout", [HEADS * DHEAD, DIM], BF16, isOutput=False)
    maskT_d = nc.declare_dram_parameter("maskT", [P, P], BF16, isOutput=False)
    ones1_d = nc.declare_dram_parameter("ones1", [1, DHEAD], BF16, isOutput=False)
    out_d = nc.declare_dram_parameter("out", [R, DIM], F32, isOutput=True)

    # token blocks for the qkT projection (moving dim <= 512)
    tok_blocks = [(b, min(512, T - b)) for b in range(0, T, 512)]

    with tile.TileContext(nc) as tc:
        with (
            tc.tile_pool(name="pers", bufs=1) as pers,
            tc.tile_pool(name="work", bufs=3) as work,
            tc.tile_pool(name="outp", bufs=2) as outp,
            tc.tile_pool(name="ps512", bufs=2, space="PSUM") as ps512,
            tc.tile_pool(name="pspt", bufs=2, space="PSUM") as pspt,
            tc.tile_pool(name="pso", bufs=2, space="PSUM") as pso,
            tc.tile_pool(name="psb", bufs=2, space="PSUM") as psb,
        ):
            # ---- phase A: load inputs -------------------------------------
            xT_sb = [pers.tile([P, T], BF16, tag=f"xT{k}", name=f"xT{k}") for k in range(DK)]
            w_sb = [pers.tile([P, 3 * HEADS * DHEAD], BF16, tag=f"w{k}", name=f"w{k}") for k in range(DK)]
            wo_sb = [pers.tile([P, DIM], BF16, tag=f"wo{m}", name=f"wo{m}") for m in range(4)]
            maskT_sb = pers.tile([P, P], BF16, tag="maskT")
            ones1_sb = pers.tile([1, DHEAD], BF16, tag="ones1")
            for k in range(DK):
                nc.sync.dma_start(xT_sb[k][:], xT_d[k * P:(k + 1) * P, :])
                nc.sync.dma_start(w_sb[k][:], wqkv_d[k * P:(k + 1) * P, :])
            for m in range(4):
                nc.sync.dma_start(wo_sb[m][:], wout_d[m * P:(m + 1) * P, :])
            nc.sync.dma_start(maskT_sb[:], maskT_d[:])
            nc.sync.dma_start(ones1_sb[:], ones1_d[:])

            # ---- phase B: qkT[m] = w_qk[:, m-chunk].T @ xT  ([128, T]) ----
            qk_sb = [pers.tile([P, T], BF16, tag=f"qk{m}", name=f"qk{m}") for m in range(8)]
            for m in range(8):
                for (b0, bw) in tok_blocks:
                    pq = ps512.tile([P, 512], F32, tag="mm512", name="mm512")
                    for k in range(DK):
                        nc.tensor.matmul(
                            pq[:, :bw],
                            lhsT=w_sb[k][:, m * P:(m + 1) * P],
                            rhs=xT_sb[k][:, b0:b0 + bw],
                            start=(k == 0), stop=(k == DK - 1),
                        )
                    nc.vector.tensor_copy(qk_sb[m][:, b0:b0 + bw], pq[:, :bw])

            # ---- phase C: v[t] = xT[:, t-tile].T @ w_v  (+ ones column) ---
            # v_sb[t] is [128 tok, 8 heads, 65]; [:, h, 0:64] = v, [:, h, 64] = 1
            v_sb = [pers.tile([P, HEADS, DHEAD + 1], BF16, tag=f"v{t}", name=f"v{t}") for t in range(NW + 1)]
            for t in range(NW + 1):
                nc.vector.memset(v_sb[t][:, :, DHEAD:DHEAD + 1], 1.0)
                pv = ps512.tile([P, 512], F32, tag="mm512", name="mm512")
                for k in range(DK):
                    nc.tensor.matmul(
                        pv[:],
                        lhsT=xT_sb[k][:, t * P:(t + 1) * P],
                        rhs=w_sb[k][:, 1024:1536],
                        start=(k == 0), stop=(k == DK - 1),
                    )
                nc.vector.tensor_copy(
                    v_sb[t][:, :, 0:DHEAD],
                    pv.rearrange("p (h d) -> p h d", h=HEADS),
                )

            # ---- phase D: attention per (window, head) --------------------
            # attn_sb[m] rows 0:64 = head 2m, 64:128 = head 2m+1 (out.T layout)
            attn_sb = [pers.tile([P, R], BF16, tag=f"at{m}", name=f"at{m}") for m in range(4)]
            for w in range(NW):
                for h in range(HEADS):
                    mq, off = h // 2, (h % 2) * 64
                    mk = 4 + h // 2
                    i0 = (w + 1) * P
                    ppt = pspt.tile([P, 2, P], F32, tag="pT", name="pT")
                    # scores transposed: pT[j, i] for j in prev/cur window
                    for jc in range(2):
                        j0 = (w + jc) * P
                        nc.tensor.matmul(
                            ppt[:, jc, :],
                            lhsT=qk_sb[mk][off:off + 64, j0:j0 + P],
                            rhs=qk_sb[mq][off:off + 64, i0:i0 + P],
                            start=True, stop=True,
                        )
                    pt_sb = work.tile([P, 2, P], BF16, tag="pt_sb", name="pt_sb")
                    nc.scalar.activation(pt_sb[:], ppt[:],
                                         mybir.ActivationFunctionType.Exp, scale=SCALE)
                    # causal mask inside the current window (prev window is
                    # fully visible: j <= i + 128 always holds there)
                    nc.vector.tensor_mul(pt_sb[:, 1, :], pt_sb[:, 1, :], maskT_sb[:])
                    # attn @ v (+ denominator in row 64, from the ones column)
                    po = pso.tile([DHEAD + 1, P], F32, tag="o", name="po")
                    for jc in range(2):
                        nc.tensor.matmul(
                            po[:],
                            lhsT=v_sb[w + jc][:, h, :],
                            rhs=pt_sb[:, jc, :],
                            start=(jc == 0), stop=(jc == 1),
                        )
                    r_sb = work.tile([1, P], BF16, tag="r_sb", name="r_sb")
                    with nc.allow_low_precision(reason="softmax denom recip in bf16"):
                        nc.vector.reciprocal(r_sb[:], po[DHEAD:DHEAD + 1, :])
                    # broadcast recip across 64 partitions via K=1 outer product
                    pb = psb.tile([DHEAD, P], F32, tag="b", name="pb")
                    nc.tensor.matmul(pb[:], lhsT=ones1_sb[:], rhs=r_sb[:],
                                     start=True, stop=True)
                    b_sb = work.tile([DHEAD, P], F32, tag="b_sb", name="b_sb")
                    nc.scalar.copy(b_sb[:], pb[:])
                    nc.vector.tensor_mul(
                        attn_sb[mq][off:off + 64, w * P:(w + 1) * P],
                        po[0:DHEAD, :], b_sb[:],
                    )

            # ---- phase E: out = attn.T @ w_out ----------------------------
            for t in range(NW):
                o_sb = outp.tile([P, DIM], F32, tag="o_sb", name="o_sb")
                for nf in range(2):
                    pf = ps512.tile([P, 512], F32, tag="mm512", name="mm512")
                    for m in range(4):
                        nc.tensor.matmul(
                            pf[:],
                            lhsT=attn_sb[m][:, t * P:(t + 1) * P],
                            rhs=wo_sb[m][:, nf * 512:(nf + 1) * 512],
                            start=(m == 0), stop=(m == 3),
                        )
                    nc.vector.tensor_copy(o_sb[:, nf * 512:(nf + 1) * 512], pf[:])
                nc.sync.dma_start(out_d[t * P:(t + 1) * P, :], o_sb[:])

    nc.compile()
    return nc


def _get_nc():
    if "nc" not in _CACHE:
        _CACHE["nc"] = _build()
    return _CACHE["nc"]


def kernel(x, w_qkv, w_out, b_out):
    x = np.asarray(x, dtype=np.float32)
    w_qkv_b = np.asarray(w_qkv, dtype=np.float32).astype(ml_dtypes.bfloat16)
    w_out_b = np.asarray(w_out, dtype=np.float32).astype(ml_dtypes.bfloat16)
    b_out = np.asarray(b_out, dtype=np.float32)

    # maskT[j, i] = 1 where j <= i  (transposed causal mask for current window)
    maskT = np.triu(np.ones((P, P), dtype=np.float32)).astype(ml_dtypes.bfloat16)
    ones1 = np.ones((1, DHEAD), dtype=ml_dtypes.bfloat16)

    x_pad = np.concatenate([np.zeros((WSZ, DIM), np.float32), x], axis=0)
    in_maps = []
    for c in range(NCORES):
        x_sh = x_pad[c * R:c * R + T]                       # (2176, 1024)
        xT = np.ascontiguousarray(x_sh.T).astype(ml_dtypes.bfloat16)
        in_maps.append({
            "xT": xT,
            "wqkv": w_qkv_b,
            "wout": w_out_b,
            "maskT": maskT,
            "ones1": ones1,
        })

    nc = _get_nc()
    res = run_bass_kernel_spmd(nc, in_maps, core_ids=list(range(NCORES)))
    out = np.concatenate([res.results[c]["out"] for c in range(NCORES)], axis=0)
    return out + b_out[None, :]



# revision 8
# speedup vs baseline: 1.4090x; 1.4090x over previous
"""Local (windowed) attention kernel for Trainium2, sequence-parallel over 8 NeuronCores.

Reference computation (fp32):
    qkv = x @ w_qkv ; q,k,v split, reshaped to (head, window, 128, 64)
    k,v get a 1-window zero-padded lookback -> (head, window, 256, 64)
    sim = q @ k.T * d^-0.5, causal-banded mask, softmax, out = attn @ v
    y = out @ w_out + b_out

Sharding: 128 windows of 128 tokens -> 16 windows per core, plus a 128-row
halo of x from the previous core (zeros for core 0, which exactly reproduces
the reference's zero-pad lookback including its effect on the softmax
denominator). No inter-core communication.

Device dataflow (per core, bf16 matmuls accumulating in fp32):
  qkT = w_qk.T @ xT keeps head features on partitions; v = xT.T @ w_v keeps
  tokens on partitions with a ones-column per head so attn@v also emits the
  softmax denominator. Scores are computed transposed (pT[j, i]) and j-batched:
  one matmul per k-window covers both q-windows that see it, with the two
  heads of a pair row-tiled onto disjoint PE row groups. attn@v accumulates
  four output windows into one PSUM bank with shingled N=256 matmuls; the
  softmax division is denom-row -> K=1 broadcast matmul -> one fused
  reciprocal_approx_fast -> one multiply, all at [64, 512] granularity so no
  lane-starved [1, N] vector work remains. Softmax skips max-subtraction
  (logits are ~N(0, 0.4); exp is safe in fp32).
"""

import sys

sys.path.insert(0, "/opt/trn_rl_repo")

import numpy as np
import ml_dtypes

import concourse.bass as bass
import concourse.mybir as mybir
import concourse.tile as tile
from concourse import bacc
from concourse.bass_utils import run_bass_kernel_spmd

BF16 = mybir.dt.bfloat16
F32 = mybir.dt.float32

N = 16384
DIM = 1024
HEADS = 8
DHEAD = 64
WSZ = 128
NCORES = 8
R = N // NCORES            # 2048 own rows per core
T = R + WSZ                # 2176 rows incl. halo
NW = R // WSZ              # 16 own windows
DK = DIM // 128            # 8 contraction chunks
P = 128
SCALE = DHEAD ** -0.5

# token blocks for the qkT projection; q skips the halo window (its queries
# are never used), k covers all 17 windows
K_BLOCKS = [(0, 512), (512, 512), (1024, 512), (1536, 512), (2048, 128)]
Q_BLOCKS = [(128, 512), (640, 512), (1152, 512), (1664, 512)]

_CACHE = {}


def _build():
    nc = bacc.Bacc()
    xT_d = nc.declare_dram_parameter("xT", [DIM, T], BF16, isOutput=False)
    wqkv_d = nc.declare_dram_parameter("wqkv", [DIM, 3 * HEADS * DHEAD], BF16, isOutput=False)
    wout_d = nc.declare_dram_parameter("wout", [HEADS * DHEAD, DIM], BF16, isOutput=False)
    maskT_d = nc.declare_dram_parameter("maskT", [P, P], BF16, isOutput=False)
    ones1_d = nc.declare_dram_parameter("ones1", [1, DHEAD], BF16, isOutput=False)
    import os
    OUT_DT = F32 if os.environ.get("OUT_F32") else BF16
    out_d = nc.declare_dram_parameter("out", [R, DIM], OUT_DT, isOutput=True)

    with tile.TileContext(nc) as tc:
        with (
            tc.tile_pool(name="pers", bufs=1) as pers,
            tc.tile_pool(name="ptp", bufs=6) as ptp,
            tc.tile_pool(name="dro", bufs=3) as dro,
            tc.tile_pool(name="b4p", bufs=3) as b4p,
            tc.tile_pool(name="outp", bufs=2) as outp,
            tc.tile_pool(name="ps512", bufs=2, space="PSUM") as ps512,
            tc.tile_pool(name="pspt", bufs=1, space="PSUM") as pspt,
            tc.tile_pool(name="pso4", bufs=2, space="PSUM") as pso4,
            tc.tile_pool(name="psb4", bufs=2, space="PSUM") as psb4,
        ):
            # ---- phase A: load inputs -------------------------------------
            maskT_sb = pers.tile([P, P], BF16, tag="maskT")
            ones1_sb = pers.tile([1, DHEAD], BF16, tag="ones1")
            nc.sync.dma_start(maskT_sb[:], maskT_d[:])
            nc.sync.dma_start(ones1_sb[:], ones1_d[:])
            xT_sb = [pers.tile([P, T], BF16, tag=f"xT{k}", name=f"xT{k}") for k in range(DK)]
            w_sb = [pers.tile([P, 3 * HEADS * DHEAD], BF16, tag=f"w{k}", name=f"w{k}") for k in range(DK)]
            wo_sb = [pers.tile([P, DIM], BF16, tag=f"wo{m}", name=f"wo{m}") for m in range(4)]
            for k in range(DK):
                nc.sync.dma_start(xT_sb[k][:], xT_d[k * P:(k + 1) * P, :])
                nc.sync.dma_start(w_sb[k][:], wqkv_d[k * P:(k + 1) * P, :])
            for m in range(4):
                nc.sync.dma_start(wo_sb[m][:], wout_d[m * P:(m + 1) * P, :])

            qk_sb = [pers.tile([P, T], BF16, tag=f"qk{m}", name=f"qk{m}") for m in range(8)]
            v_sb = [pers.tile([P, HEADS, DHEAD + 1], BF16, tag=f"v{t}", name=f"v{t}") for t in range(NW + 1)]
            attn_sb = [pers.tile([P, R], BF16, tag=f"at{m}", name=f"at{m}") for m in range(4)]

            def proj_qk(m):
                """qk_sb[m] = w_qkv[:, m-chunk].T @ xT  ([128, T])"""
                blocks = Q_BLOCKS if m < 4 else K_BLOCKS
                for (b0, bw) in blocks:
                    pq = ps512.tile([P, 512], F32, tag="mm512", name="mm512")
                    for k in range(DK):
                        nc.tensor.matmul(
                            pq[:, :bw],
                            lhsT=w_sb[k][:, m * P:(m + 1) * P],
                            rhs=xT_sb[k][:, b0:b0 + bw],
                            start=(k == 0), stop=(k == DK - 1),
                        )
                    nc.vector.tensor_copy(qk_sb[m][:, b0:b0 + bw], pq[:, :bw])

            def proj_v():
                """v_sb[t] = xT[:, t-tile].T @ w_v, plus a ones column per head."""
                for t in range(NW + 1):
                    nc.vector.memset(v_sb[t][:, :, DHEAD:DHEAD + 1], 1.0)
                    pv = ps512.tile([P, 512], F32, tag="mm512", name="mm512")
                    for k in range(DK):
                        nc.tensor.matmul(
                            pv[:],
                            lhsT=xT_sb[k][:, t * P:(t + 1) * P],
                            rhs=w_sb[k][:, 1024:1536],
                            start=(k == 0), stop=(k == DK - 1),
                        )
                    nc.vector.tensor_copy(
                        v_sb[t][:, :, 0:DHEAD],
                        pv.rearrange("p (h d) -> p h d", h=HEADS),
                    )

            # the two heads of a pair are row-tiled onto disjoint PE row
            # groups, so their score matmuls execute CONCURRENTLY — each head
            # must drain into its own PSUM bank (same-bank concurrent drains
            # are a fatal HW collision). Two persistent double-slot banks.
            ppt_hh = [pspt.tile([P, 2, 256], F32, tag=f"pT{hh}", name=f"pT{hh}")
                      for hh in range(2)]

            def scores(pr, t, pt_tiles):
                """j-batched transposed scores for k-window t, heads 2pr/2pr+1.

                ppt[:, 0:128]   = k_t . q_t     (current window, causal mask)
                ppt[:, 128:256] = k_t . q_{t+1} (t is its lookback window)
                """
                mq, mk = pr, 4 + pr
                lo = 128 if t == 0 else 0
                hi = 128 if t == NW else 256
                slot = t % 2
                for hh in range(2):
                    off = hh * 64
                    nc.tensor.matmul(
                        ppt_hh[hh][:, slot, lo:hi],
                        lhsT=qk_sb[mk][off:off + 64, t * P:(t + 1) * P],
                        rhs=qk_sb[mq][off:off + 64, t * P + lo:t * P + hi],
                        start=True, stop=True,
                    )
                pt = ptp.tile([P, 2, 256], BF16, tag="pt", name="pt")
                for hh in range(2):
                    nc.scalar.activation(pt[:, hh, lo:hi], ppt_hh[hh][:, slot, lo:hi],
                                         mybir.ActivationFunctionType.Exp, scale=SCALE)
                    if t > 0:
                        nc.vector.tensor_mul(pt[:, hh, 0:P], pt[:, hh, 0:P], maskT_sb[:])
                pt_tiles[t] = pt

            def epilogue(pr, g, pt_tiles):
                """attn@v + softmax divide for windows 4g..4g+3, heads 2pr/2pr+1."""
                mq = pr
                for hh in range(2):
                    off = hh * 64
                    po4 = pso4.tile([DHEAD + 1, 512], F32, tag="po4", name="po4")
                    # window w = 4g+i in cols i*128:(i+1)*128: lookback window
                    # contribution (v_w, ptT_w[:, 128:256]) then current-window
                    # (v_{w+1}, ptT_{w+1}[:, 0:128]); each matmul targets one
                    # uniform 128-col slice of the shared bank
                    for i in range(4):
                        t0 = 4 * g + i
                        nc.tensor.matmul(
                            po4[:, i * P:(i + 1) * P],
                            lhsT=v_sb[t0][:, 2 * pr + hh, :],
                            rhs=pt_tiles[t0][:, hh, 128:256],
                            start=True, stop=False,
                        )
                        nc.tensor.matmul(
                            po4[:, i * P:(i + 1) * P],
                            lhsT=v_sb[t0 + 1][:, 2 * pr + hh, :],
                            rhs=pt_tiles[t0 + 1][:, hh, 0:128],
                            start=False, stop=True,
                        )
                    # softmax denominators rode along in row 64 (ones column)
                    d4 = dro.tile([1, 512], BF16, tag="d4", name="d4")
                    nc.scalar.copy(d4[:], po4[DHEAD:DHEAD + 1, :])
                    pb4 = psb4.tile([DHEAD, 512], F32, tag="pb4", name="pb4")
                    nc.tensor.matmul(pb4[:], lhsT=ones1_sb[:], rhs=d4[:],
                                     start=True, stop=True)
                    b4 = b4p.tile([DHEAD, 512], F32, tag="b4", name="b4")
                    import os
                    if os.environ.get("LEGACY_RECIP"):
                        nc.vector.reciprocal(b4[:], pb4[:])
                    else:
                        nc.vector.reciprocal_approx_fast(b4[:], pb4[:])
                    nc.vector.tensor_mul(
                        attn_sb[mq][off:off + 64, g * 512:(g + 1) * 512],
                        po4[0:DHEAD, :], b4[:],
                    )

            def attention(pr):
                pt_tiles = {}
                scores(pr, 0, pt_tiles)
                for g in range(4):
                    for t in range(4 * g + 1, 4 * g + 5):
                        scores(pr, t, pt_tiles)
                    epilogue(pr, g, pt_tiles)

            def out_proj():
                for t in range(NW):
                    o_sb = outp.tile([P, DIM], OUT_DT, tag="o_sb", name="o_sb")
                    for nf in range(2):
                        pf = ps512.tile([P, 512], F32, tag="mm512", name="mm512")
                        for m in range(4):
                            nc.tensor.matmul(
                                pf[:],
                                lhsT=attn_sb[m][:, t * P:(t + 1) * P],
                                rhs=wo_sb[m][:, nf * 512:(nf + 1) * 512],
                                start=(m == 0), stop=(m == 3),
                            )
                        nc.scalar.copy(o_sb[:, nf * 512:(nf + 1) * 512], pf[:])
                    nc.sync.dma_start(out_d[t * P:(t + 1) * P, :], o_sb[:])

            # issue order: interleave projections with attention head-pairs so
            # the PE always has dense matmul work while ACT/DVE chew on the
            # softmax chain of the previous head-pair
            proj_qk(0)
            proj_qk(4)
            proj_v()
            attention(0)
            for pr in range(1, 4):
                proj_qk(pr)
                proj_qk(4 + pr)
                attention(pr)
            out_proj()

    nc.compile()
    return nc


def _get_nc():
    if "nc" not in _CACHE:
        _CACHE["nc"] = _build()
    return _CACHE["nc"]


def kernel(x, w_qkv, w_out, b_out):
    x = np.asarray(x, dtype=np.float32)
    w_qkv_b = np.asarray(w_qkv, dtype=np.float32).astype(ml_dtypes.bfloat16)
    w_out_b = np.asarray(w_out, dtype=np.float32).astype(ml_dtypes.bfloat16)
    b_out = np.asarray(b_out, dtype=np.float32)

    # maskT[j, i] = 1 where j <= i  (transposed causal mask for current window)
    maskT = np.triu(np.ones((P, P), dtype=np.float32)).astype(ml_dtypes.bfloat16)
    ones1 = np.ones((1, DHEAD), dtype=ml_dtypes.bfloat16)

    x_pad = np.concatenate([np.zeros((WSZ, DIM), np.float32), x], axis=0)
    in_maps = []
    for c in range(NCORES):
        x_sh = x_pad[c * R:c * R + T]                       # (2176, 1024)
        xT = np.ascontiguousarray(x_sh.T).astype(ml_dtypes.bfloat16)
        in_maps.append({
            "xT": xT,
            "wqkv": w_qkv_b,
            "wout": w_out_b,
            "maskT": maskT,
            "ones1": ones1,
        })

    nc = _get_nc()
    res = run_bass_kernel_spmd(nc, in_maps, core_ids=list(range(NCORES)))
    out = np.concatenate(
        [np.asarray(res.results[c]["out"]).astype(np.float32) for c in range(NCORES)],
        axis=0,
    )
    return out + b_out[None, :]


# revision 10
# speedup vs baseline: 1.4275x; 1.0131x over previous
"""Local (windowed) attention kernel for Trainium2, sequence-parallel over 8 NeuronCores.

Reference computation (fp32):
    qkv = x @ w_qkv ; q,k,v split, reshaped to (head, window, 128, 64)
    k,v get a 1-window zero-padded lookback -> (head, window, 256, 64)
    sim = q @ k.T * d^-0.5, causal-banded mask, softmax, out = attn @ v
    y = out @ w_out + b_out

Sharding: 128 windows of 128 tokens -> 16 windows per core, plus a 128-row
halo of x from the previous core (zeros for core 0, which exactly reproduces
the reference's zero-pad lookback including its effect on the softmax
denominator). No inter-core communication.

Device dataflow (per core, bf16 matmuls accumulating in fp32):
  qkT = w_qk.T @ xT keeps head features on partitions; v = xT.T @ w_v keeps
  tokens on partitions with a ones-column per head so attn@v also emits the
  softmax denominator. Scores are computed transposed (pT[j, i]) and j-batched:
  one matmul per k-window covers both q-windows that see it, with the two
  heads of a pair row-tiled onto disjoint PE row groups. attn@v accumulates
  four output windows into one PSUM bank with shingled N=256 matmuls; the
  softmax division is denom-row -> K=1 broadcast matmul -> one fused
  reciprocal_approx_fast -> one multiply, all at [64, 512] granularity so no
  lane-starved [1, N] vector work remains. Softmax skips max-subtraction
  (logits are ~N(0, 0.4); exp is safe in fp32).
"""

import sys

sys.path.insert(0, "/opt/trn_rl_repo")

import numpy as np
import ml_dtypes

import concourse.bass as bass
import concourse.mybir as mybir
import concourse.tile as tile
from concourse import bacc
from concourse.bass_utils import run_bass_kernel_spmd

BF16 = mybir.dt.bfloat16
F32 = mybir.dt.float32

N = 16384
DIM = 1024
HEADS = 8
DHEAD = 64
WSZ = 128
NCORES = 8
R = N // NCORES            # 2048 own rows per core
T = R + WSZ                # 2176 rows incl. halo
NW = R // WSZ              # 16 own windows
DK = DIM // 128            # 8 contraction chunks
P = 128
SCALE = DHEAD ** -0.5

# token blocks for the qkT projection; q skips the halo window (its queries
# are never used), k covers all 17 windows
K_BLOCKS = [(0, 512), (512, 512), (1024, 512), (1536, 512), (2048, 128)]
Q_BLOCKS = [(128, 512), (640, 512), (1152, 512), (1664, 512)]

_CACHE = {}


def _build():
    nc = bacc.Bacc()
    xT_d = nc.declare_dram_parameter("xT", [DIM, T], BF16, isOutput=False)
    wqkv_d = nc.declare_dram_parameter("wqkv", [DIM, 3 * HEADS * DHEAD], BF16, isOutput=False)
    wout_d = nc.declare_dram_parameter("wout", [HEADS * DHEAD, DIM], BF16, isOutput=False)
    maskT_d = nc.declare_dram_parameter("maskT", [P, P], BF16, isOutput=False)
    ones1_d = nc.declare_dram_parameter("ones1", [1, DHEAD], BF16, isOutput=False)
    import os
    OUT_DT = F32 if os.environ.get("OUT_F32") else BF16
    out_d = nc.declare_dram_parameter("out", [R, DIM], OUT_DT, isOutput=True)

    with tile.TileContext(nc) as tc:
        with (
            tc.tile_pool(name="pers", bufs=1) as pers,
            tc.tile_pool(name="ptp", bufs=6) as ptp,
            tc.tile_pool(name="dro", bufs=3) as dro,
            tc.tile_pool(name="b4p", bufs=3) as b4p,
            tc.tile_pool(name="outp", bufs=2) as outp,
            tc.tile_pool(name="ps512", bufs=3, space="PSUM") as ps512,
            tc.tile_pool(name="pspt", bufs=1, space="PSUM") as pspt,
            tc.tile_pool(name="pso4", bufs=2, space="PSUM") as pso4,
            tc.tile_pool(name="psb4", bufs=1, space="PSUM") as psb4,
        ):
            # ---- phase A: load inputs -------------------------------------
            # few large strided DMAs (DIRECT2D issue on the sync sequencer
            # costs ~650ns each, so 22 small loads would serialize ~14us);
            # k-chunk pairs land progressively so the k-accumulation loops in
            # the projections can start before the full tensor arrives
            maskT_sb = pers.tile([P, P], BF16, tag="maskT")
            ones1_sb = pers.tile([1, DHEAD], BF16, tag="ones1")
            nc.sync.dma_start(maskT_sb[:], maskT_d[:])
            nc.sync.dma_start(ones1_sb[:], ones1_d[:])
            xTall = pers.tile([P, DK, T], BF16, tag="xTall")
            wall = pers.tile([P, DK, 3 * HEADS * DHEAD], BF16, tag="wall")
            woall = pers.tile([P, 4, DIM], BF16, tag="woall")
            xT_dr = xT_d.rearrange("(k p) t -> p k t", p=P)
            wqkv_dr = wqkv_d.rearrange("(k p) c -> p k c", p=P)
            wout_dr = wout_d.rearrange("(m p) c -> p m c", p=P)
            for kp in range(4):
                nc.sync.dma_start(xTall[:, 2 * kp:2 * kp + 2, :], xT_dr[:, 2 * kp:2 * kp + 2, :])
                nc.sync.dma_start(wall[:, 2 * kp:2 * kp + 2, :], wqkv_dr[:, 2 * kp:2 * kp + 2, :])
            nc.sync.dma_start(woall[:], wout_dr[:])

            qk_sb = [pers.tile([P, T], BF16, tag=f"qk{m}", name=f"qk{m}") for m in range(8)]
            v_sb = [pers.tile([P, HEADS, DHEAD + 1], BF16, tag=f"v{t}", name=f"v{t}") for t in range(NW + 1)]
            attn_sb = [pers.tile([P, R], BF16, tag=f"at{m}", name=f"at{m}") for m in range(4)]

            def proj_qk(m):
                """qk_sb[m] = w_qkv[:, m-chunk].T @ xT  ([128, T])"""
                blocks = Q_BLOCKS if m < 4 else K_BLOCKS
                for (b0, bw) in blocks:
                    pq = ps512.tile([P, 512], F32, tag="mm512", name="mm512")
                    for k in range(DK):
                        nc.tensor.matmul(
                            pq[:, :bw],
                            lhsT=wall[:, k, m * P:(m + 1) * P],
                            rhs=xTall[:, k, b0:b0 + bw],
                            start=(k == 0), stop=(k == DK - 1),
                        )
                    nc.vector.tensor_copy(qk_sb[m][:, b0:b0 + bw], pq[:, :bw])

            def proj_v():
                """v_sb[t] = xT[:, t-tile].T @ w_v, plus a ones column per head."""
                for t in range(NW + 1):
                    nc.vector.memset(v_sb[t][:, :, DHEAD:DHEAD + 1], 1.0)
                    pv = ps512.tile([P, 512], F32, tag="mm512", name="mm512")
                    for k in range(DK):
                        nc.tensor.matmul(
                            pv[:],
                            lhsT=xTall[:, k, t * P:(t + 1) * P],
                            rhs=wall[:, k, 1024:1536],
                            start=(k == 0), stop=(k == DK - 1),
                        )
                    nc.vector.tensor_copy(
                        v_sb[t][:, :, 0:DHEAD],
                        pv.rearrange("p (h d) -> p h d", h=HEADS),
                    )

            # the two heads of a pair are row-tiled onto disjoint PE row
            # groups, so their score matmuls execute CONCURRENTLY — each head
            # must drain into its own PSUM bank (same-bank concurrent drains
            # are a fatal HW collision). Two persistent double-slot banks.
            ppt_hh = [pspt.tile([P, 2, 256], F32, tag=f"pT{hh}", name=f"pT{hh}")
                      for hh in range(2)]

            def scores(pr, t, pt_tiles):
                """j-batched transposed scores for k-window t, heads 2pr/2pr+1.

                ppt[:, 0:128]   = k_t . q_t     (current window, causal mask)
                ppt[:, 128:256] = k_t . q_{t+1} (t is its lookback window)
                """
                mq, mk = pr, 4 + pr
                lo = 128 if t == 0 else 0
                hi = 128 if t == NW else 256
                slot = t % 2
                for hh in range(2):
                    off = hh * 64
                    nc.tensor.matmul(
                        ppt_hh[hh][:, slot, lo:hi],
                        lhsT=qk_sb[mk][off:off + 64, t * P:(t + 1) * P],
                        rhs=qk_sb[mq][off:off + 64, t * P + lo:t * P + hi],
                        start=True, stop=True,
                    )
                pt = ptp.tile([P, 2, 256], BF16, tag="pt", name="pt")
                for hh in range(2):
                    nc.scalar.activation(pt[:, hh, lo:hi], ppt_hh[hh][:, slot, lo:hi],
                                         mybir.ActivationFunctionType.Exp, scale=SCALE)
                    if t > 0:
                        nc.vector.tensor_mul(pt[:, hh, 0:P], pt[:, hh, 0:P], maskT_sb[:])
                pt_tiles[t] = pt

            def epilogue(pr, g, pt_tiles):
                """attn@v + softmax divide for windows 4g..4g+3, heads 2pr/2pr+1."""
                mq = pr
                for hh in range(2):
                    off = hh * 64
                    po4 = pso4.tile([DHEAD + 1, 512], F32, tag="po4", name="po4")
                    # window w = 4g+i in cols i*128:(i+1)*128: lookback window
                    # contribution (v_w, ptT_w[:, 128:256]) then current-window
                    # (v_{w+1}, ptT_{w+1}[:, 0:128]); each matmul targets one
                    # uniform 128-col slice of the shared bank
                    for i in range(4):
                        t0 = 4 * g + i
                        nc.tensor.matmul(
                            po4[:, i * P:(i + 1) * P],
                            lhsT=v_sb[t0][:, 2 * pr + hh, :],
                            rhs=pt_tiles[t0][:, hh, 128:256],
                            start=True, stop=False,
                        )
                        nc.tensor.matmul(
                            po4[:, i * P:(i + 1) * P],
                            lhsT=v_sb[t0 + 1][:, 2 * pr + hh, :],
                            rhs=pt_tiles[t0 + 1][:, hh, 0:128],
                            start=False, stop=True,
                        )
                    # softmax denominators rode along in row 64 (ones column)
                    d4 = dro.tile([1, 512], BF16, tag="d4", name="d4")
                    nc.scalar.copy(d4[:], po4[DHEAD:DHEAD + 1, :])
                    pb4 = psb4.tile([DHEAD, 512], F32, tag="pb4", name="pb4")
                    nc.tensor.matmul(pb4[:], lhsT=ones1_sb[:], rhs=d4[:],
                                     start=True, stop=True)
                    b4 = b4p.tile([DHEAD, 512], F32, tag="b4", name="b4")
                    import os
                    if os.environ.get("LEGACY_RECIP"):
                        nc.vector.reciprocal(b4[:], pb4[:])
                    else:
                        nc.vector.reciprocal_approx_fast(b4[:], pb4[:])
                    nc.vector.tensor_mul(
                        attn_sb[mq][off:off + 64, g * 512:(g + 1) * 512],
                        po4[0:DHEAD, :], b4[:],
                    )

            def attention(pr):
                pt_tiles = {}
                scores(pr, 0, pt_tiles)
                for g in range(4):
                    for t in range(4 * g + 1, 4 * g + 5):
                        scores(pr, t, pt_tiles)
                    epilogue(pr, g, pt_tiles)

            def out_proj():
                for t in range(NW):
                    o_sb = outp.tile([P, DIM], OUT_DT, tag="o_sb", name="o_sb")
                    for nf in range(2):
                        pf = ps512.tile([P, 512], F32, tag="mm512", name="mm512")
                        for m in range(4):
                            nc.tensor.matmul(
                                pf[:],
                                lhsT=attn_sb[m][:, t * P:(t + 1) * P],
                                rhs=woall[:, m, nf * 512:(nf + 1) * 512],
                                start=(m == 0), stop=(m == 3),
                            )
                        nc.scalar.copy(o_sb[:, nf * 512:(nf + 1) * 512], pf[:])
                    nc.sync.dma_start(out_d[t * P:(t + 1) * P, :], o_sb[:])

            # issue order: interleave projections with attention head-pairs so
            # the PE always has dense matmul work while ACT/DVE chew on the
            # softmax chain of the previous head-pair
            proj_qk(0)
            proj_qk(4)
            proj_v()
            attention(0)
            for pr in range(1, 4):
                proj_qk(pr)
                proj_qk(4 + pr)
                attention(pr)
            out_proj()

    nc.compile()
    return nc


def _get_nc():
    if "nc" not in _CACHE:
        _CACHE["nc"] = _build()
    return _CACHE["nc"]


def kernel(x, w_qkv, w_out, b_out):
    x = np.asarray(x, dtype=np.float32)
    w_qkv_b = np.asarray(w_qkv, dtype=np.float32).astype(ml_dtypes.bfloat16)
    w_out_b = np.asarray(w_out, dtype=np.float32).astype(ml_dtypes.bfloat16)
    b_out = np.asarray(b_out, dtype=np.float32)

    # maskT[j, i] = 1 where j <= i  (transposed causal mask for current window)
    maskT = np.triu(np.ones((P, P), dtype=np.float32)).astype(ml_dtypes.bfloat16)
    ones1 = np.ones((1, DHEAD), dtype=ml_dtypes.bfloat16)

    x_pad = np.concatenate([np.zeros((WSZ, DIM), np.float32), x], axis=0)
    in_maps = []
    for c in range(NCORES):
        x_sh = x_pad[c * R:c * R + T]                       # (2176, 1024)
        xT = np.ascontiguousarray(x_sh.T).astype(ml_dtypes.bfloat16)
        in_maps.append({
            "xT": xT,
            "wqkv": w_qkv_b,
            "wout": w_out_b,
            "maskT": maskT,
            "ones1": ones1,
        })

    nc = _get_nc()
    res = run_bass_kernel_spmd(nc, in_maps, core_ids=list(range(NCORES)))
    out = np.concatenate(
        [np.asarray(res.results[c]["out"]).astype(np.float32) for c in range(NCORES)],
        axis=0,
    )
    return out + b_out[None, :]


# revision 12
# speedup vs baseline: 1.4925x; 1.0455x over previous
"""Local (windowed) attention kernel for Trainium2, sequence-parallel over 8 NeuronCores.

Reference computation (fp32):
    qkv = x @ w_qkv ; q,k,v split, reshaped to (head, window, 128, 64)
    k,v get a 1-window zero-padded lookback -> (head, window, 256, 64)
    sim = q @ k.T * d^-0.5, causal-banded mask, softmax, out = attn @ v
    y = out @ w_out + b_out

Sharding: 128 windows of 128 tokens -> 16 windows per core, plus a 128-row
halo of x from the previous core (zeros for core 0, which exactly reproduces
the reference's zero-pad lookback including its effect on the softmax
denominator). No inter-core communication.

Device dataflow (per core, bf16 matmuls accumulating in fp32):
  qkT = w_qk.T @ xT keeps head features on partitions; v = xT.T @ w_v keeps
  tokens on partitions with a ones-column per head so attn@v also emits the
  softmax denominator. Scores are computed transposed (pT[j, i]) and j-batched:
  one matmul per k-window covers both q-windows that see it, with the two
  heads of a pair row-tiled onto disjoint PE row groups. attn@v accumulates
  four output windows into one PSUM bank with shingled N=256 matmuls; the
  softmax division is denom-row -> K=1 broadcast matmul -> one fused
  reciprocal_approx_fast -> one multiply, all at [64, 512] granularity so no
  lane-starved [1, N] vector work remains. Softmax skips max-subtraction
  (logits are ~N(0, 0.4); exp is safe in fp32).
"""

import sys

sys.path.insert(0, "/opt/trn_rl_repo")

import numpy as np
import ml_dtypes

import concourse.bass as bass
import concourse.mybir as mybir
import concourse.tile as tile
from concourse import bacc
from concourse.bass_utils import run_bass_kernel_spmd

BF16 = mybir.dt.bfloat16
F32 = mybir.dt.float32

N = 16384
DIM = 1024
HEADS = 8
DHEAD = 64
WSZ = 128
NCORES = 8
R = N // NCORES            # 2048 own rows per core
T = R + WSZ                # 2176 rows incl. halo
NW = R // WSZ              # 16 own windows
DK = DIM // 128            # 8 contraction chunks
P = 128
SCALE = DHEAD ** -0.5

# token blocks for the qkT projection; q skips the halo window (its queries
# are never used), k covers all 17 windows
K_BLOCKS = [(0, 512), (512, 512), (1024, 512), (1536, 512), (2048, 128)]
Q_BLOCKS = [(128, 512), (640, 512), (1152, 512), (1664, 512)]

_CACHE = {}


def _build():
    nc = bacc.Bacc()
    xT_d = nc.declare_dram_parameter("xT", [DIM, T], BF16, isOutput=False)
    wqkv_d = nc.declare_dram_parameter("wqkv", [DIM, 3 * HEADS * DHEAD], BF16, isOutput=False)
    wout_d = nc.declare_dram_parameter("wout", [HEADS * DHEAD, DIM], BF16, isOutput=False)
    maskT_d = nc.declare_dram_parameter("maskT", [P, P], BF16, isOutput=False)
    ones1_d = nc.declare_dram_parameter("ones1", [1, DHEAD], BF16, isOutput=False)
    import os
    OUT_DT = F32 if os.environ.get("OUT_F32") else BF16
    out_d = nc.declare_dram_parameter("out", [R, DIM], OUT_DT, isOutput=True)

    with tile.TileContext(nc) as tc:
        with (
            tc.tile_pool(name="pers", bufs=1) as pers,
            tc.tile_pool(name="ptp", bufs=6) as ptp,
            tc.tile_pool(name="dro", bufs=3) as dro,
            tc.tile_pool(name="b4p", bufs=3) as b4p,
            tc.tile_pool(name="outp", bufs=2) as outp,
            tc.tile_pool(name="ps512", bufs=3, space="PSUM") as ps512,
            tc.tile_pool(name="pspt", bufs=1, space="PSUM") as pspt,
            tc.tile_pool(name="pso4", bufs=2, space="PSUM") as pso4,
            tc.tile_pool(name="psb4", bufs=1, space="PSUM") as psb4,
        ):
            # ---- phase A: load inputs -------------------------------------
            # per-chunk DMAs, issue split across the two HWDGE rings (sync +
            # scalar sequencers) so the ~650ns DIRECT2D issue costs overlap;
            # chunk k of xT and w_qkv land together so the k-accumulation
            # loops in the projections start before the full tensors arrive
            maskT_sb = pers.tile([P, P], BF16, tag="maskT")
            ones1_sb = pers.tile([1, DHEAD], BF16, tag="ones1")
            nc.scalar.dma_start(maskT_sb[:], maskT_d[:])
            nc.scalar.dma_start(ones1_sb[:], ones1_d[:])
            xTall = pers.tile([P, DK, T], BF16, tag="xTall")
            wall = pers.tile([P, DK, 3 * HEADS * DHEAD], BF16, tag="wall")
            woall = pers.tile([P, 4, DIM], BF16, tag="woall")
            xT_dr = xT_d.rearrange("(k p) t -> p k t", p=P)
            wqkv_dr = wqkv_d.rearrange("(k p) c -> p k c", p=P)
            wout_dr = wout_d.rearrange("(m p) c -> p m c", p=P)
            for k in range(DK):
                nc.sync.dma_start(xTall[:, k, :], xT_dr[:, k, :])
                nc.scalar.dma_start(wall[:, k, :], wqkv_dr[:, k, :])
            for m in range(4):
                nc.scalar.dma_start(woall[:, m, :], wout_dr[:, m, :])

            qk_sb = [pers.tile([P, T], BF16, tag=f"qk{m}", name=f"qk{m}") for m in range(8)]
            v_sb = [pers.tile([P, HEADS, DHEAD + 1], BF16, tag=f"v{t}", name=f"v{t}") for t in range(NW + 1)]
            attn_sb = [pers.tile([P, R], BF16, tag=f"at{m}", name=f"at{m}") for m in range(4)]

            def proj_qk(m):
                """qk_sb[m] = w_qkv[:, m-chunk].T @ xT  ([128, T])"""
                blocks = Q_BLOCKS if m < 4 else K_BLOCKS
                for (b0, bw) in blocks:
                    pq = ps512.tile([P, 512], F32, tag="mm512", name="mm512")
                    for k in range(DK):
                        nc.tensor.matmul(
                            pq[:, :bw],
                            lhsT=wall[:, k, m * P:(m + 1) * P],
                            rhs=xTall[:, k, b0:b0 + bw],
                            start=(k == 0), stop=(k == DK - 1),
                        )
                    nc.vector.tensor_copy(qk_sb[m][:, b0:b0 + bw], pq[:, :bw])

            def proj_v():
                """v_sb[t] = xT[:, t-tile].T @ w_v, plus a ones column per head."""
                for t in range(NW + 1):
                    nc.vector.memset(v_sb[t][:, :, DHEAD:DHEAD + 1], 1.0)
                    pv = ps512.tile([P, 512], F32, tag="mm512", name="mm512")
                    for k in range(DK):
                        nc.tensor.matmul(
                            pv[:],
                            lhsT=xTall[:, k, t * P:(t + 1) * P],
                            rhs=wall[:, k, 1024:1536],
                            start=(k == 0), stop=(k == DK - 1),
                        )
                    nc.vector.tensor_copy(
                        v_sb[t][:, :, 0:DHEAD],
                        pv.rearrange("p (h d) -> p h d", h=HEADS),
                    )

            # the two heads of a pair are row-tiled onto disjoint PE row
            # groups, so their score matmuls execute CONCURRENTLY — each head
            # must drain into its own PSUM bank (same-bank concurrent drains
            # are a fatal HW collision). Two persistent double-slot banks.
            ppt_hh = [pspt.tile([P, 2, 256], F32, tag=f"pT{hh}", name=f"pT{hh}")
                      for hh in range(2)]

            def scores(pr, t, pt_tiles):
                """j-batched transposed scores for k-window t, heads 2pr/2pr+1.

                ppt[:, 0:128]   = k_t . q_t     (current window, causal mask)
                ppt[:, 128:256] = k_t . q_{t+1} (t is its lookback window)
                """
                mq, mk = pr, 4 + pr
                lo = 128 if t == 0 else 0
                hi = 128 if t == NW else 256
                slot = t % 2
                for hh in range(2):
                    off = hh * 64
                    nc.tensor.matmul(
                        ppt_hh[hh][:, slot, lo:hi],
                        lhsT=qk_sb[mk][off:off + 64, t * P:(t + 1) * P],
                        rhs=qk_sb[mq][off:off + 64, t * P + lo:t * P + hi],
                        start=True, stop=True,
                    )
                pt = ptp.tile([P, 2, 256], BF16, tag="pt", name="pt")
                for hh in range(2):
                    nc.scalar.activation(pt[:, hh, lo:hi], ppt_hh[hh][:, slot, lo:hi],
                                         mybir.ActivationFunctionType.Exp, scale=SCALE)
                    if t > 0:
                        nc.vector.tensor_mul(pt[:, hh, 0:P], pt[:, hh, 0:P], maskT_sb[:])
                pt_tiles[t] = pt

            def epilogue(pr, g, pt_tiles):
                """attn@v + softmax divide for windows 4g..4g+3, heads 2pr/2pr+1."""
                mq = pr
                for hh in range(2):
                    off = hh * 64
                    po4 = pso4.tile([DHEAD + 1, 512], F32, tag="po4", name="po4")
                    # window w = 4g+i in cols i*128:(i+1)*128: lookback window
                    # contribution (v_w, ptT_w[:, 128:256]) then current-window
                    # (v_{w+1}, ptT_{w+1}[:, 0:128]); each matmul targets one
                    # uniform 128-col slice of the shared bank
                    for i in range(4):
                        t0 = 4 * g + i
                        nc.tensor.matmul(
                            po4[:, i * P:(i + 1) * P],
                            lhsT=v_sb[t0][:, 2 * pr + hh, :],
                            rhs=pt_tiles[t0][:, hh, 128:256],
                            start=True, stop=False,
                        )
                        nc.tensor.matmul(
                            po4[:, i * P:(i + 1) * P],
                            lhsT=v_sb[t0 + 1][:, 2 * pr + hh, :],
                            rhs=pt_tiles[t0 + 1][:, hh, 0:128],
                            start=False, stop=True,
                        )
                    # softmax denominators rode along in row 64 (ones column)
                    d4 = dro.tile([1, 512], BF16, tag="d4", name="d4")
                    nc.scalar.copy(d4[:], po4[DHEAD:DHEAD + 1, :])
                    pb4 = psb4.tile([DHEAD, 512], F32, tag="pb4", name="pb4")
                    nc.tensor.matmul(pb4[:], lhsT=ones1_sb[:], rhs=d4[:],
                                     start=True, stop=True)
                    b4 = b4p.tile([DHEAD, 512], F32, tag="b4", name="b4")
                    import os
                    if os.environ.get("LEGACY_RECIP"):
                        nc.vector.reciprocal(b4[:], pb4[:])
                    else:
                        nc.vector.reciprocal_approx_fast(b4[:], pb4[:])
                    nc.vector.tensor_mul(
                        attn_sb[mq][off:off + 64, g * 512:(g + 1) * 512],
                        po4[0:DHEAD, :], b4[:],
                    )

            def out_window(t):
                o_sb = outp.tile([P, DIM], OUT_DT, tag="o_sb", name="o_sb")
                for nf in range(2):
                    pf = ps512.tile([P, 512], F32, tag="mm512", name="mm512")
                    for m in range(4):
                        nc.tensor.matmul(
                            pf[:],
                            lhsT=attn_sb[m][:, t * P:(t + 1) * P],
                            rhs=woall[:, m, nf * 512:(nf + 1) * 512],
                            start=(m == 0), stop=(m == 3),
                        )
                    nc.scalar.copy(o_sb[:, nf * 512:(nf + 1) * 512], pf[:])
                nc.sync.dma_start(out_d[t * P:(t + 1) * P, :], o_sb[:])

            def attention(pr, emit_out=False):
                pt_tiles = {}
                scores(pr, 0, pt_tiles)
                for g in range(4):
                    for t in range(4 * g + 1, 4 * g + 5):
                        scores(pr, t, pt_tiles)
                    epilogue(pr, g, pt_tiles)
                    if emit_out:
                        for t in range(4 * g, 4 * g + 4):
                            out_window(t)

            # issue order: interleave projections with attention head-pairs so
            # the PE always has dense matmul work while ACT/DVE chew on the
            # softmax chain of the previous head-pair
            proj_qk(0)
            proj_qk(4)
            proj_v()
            attention(0)
            for pr in range(1, 3):
                proj_qk(pr)
                proj_qk(4 + pr)
                attention(pr)
            proj_qk(3)
            proj_qk(7)
            attention(3, emit_out=True)

    nc.compile()
    return nc


def _get_nc():
    if "nc" not in _CACHE:
        _CACHE["nc"] = _build()
    return _CACHE["nc"]


def kernel(x, w_qkv, w_out, b_out):
    x = np.asarray(x, dtype=np.float32)
    w_qkv_b = np.asarray(w_qkv, dtype=np.float32).astype(ml_dtypes.bfloat16)
    w_out_b = np.asarray(w_out, dtype=np.float32).astype(ml_dtypes.bfloat16)
    b_out = np.asarray(b_out, dtype=np.float32)

    # maskT[j, i] = 1 where j <= i  (transposed causal mask for current window)
    maskT = np.triu(np.ones((P, P), dtype=np.float32)).astype(ml_dtypes.bfloat16)
    ones1 = np.ones((1, DHEAD), dtype=ml_dtypes.bfloat16)

    x_pad = np.concatenate([np.zeros((WSZ, DIM), np.float32), x], axis=0)
    in_maps = []
    for c in range(NCORES):
        x_sh = x_pad[c * R:c * R + T]                       # (2176, 1024)
        xT = np.ascontiguousarray(x_sh.T).astype(ml_dtypes.bfloat16)
        in_maps.append({
            "xT": xT,
            "wqkv": w_qkv_b,
            "wout": w_out_b,
            "maskT": maskT,
            "ones1": ones1,
        })

    nc = _get_nc()
    res = run_bass_kernel_spmd(nc, in_maps, core_ids=list(range(NCORES)))
    out = np.concatenate(
        [np.asarray(res.results[c]["out"]).astype(np.float32) for c in range(NCORES)],
        axis=0,
    )
    return out + b_out[None, :]
